# revision 1
# baseline (speedup 1.0000x reference)
"""MoE kernel v3: pair-wise F-split (2 cores per expert pair, F/2 each).

Experts are paired largest-with-smallest; the pair's two cores each hold
the F-half of BOTH experts (128 KB/partition, same as v1) and process all
tokens of both experts on their half. Slot capacities are global
(CA = largest expert count, CB = largest count among the 4 "small" slot
experts), so the program is SPMD; per-core data decides which experts a
core serves. Partial outputs (bf16) from the two cores of a pair are
summed on host, then combined/scattered as in v1.

Per-core PE work: (CA + CB) columns x 256 cycles — ~4% less than v1's
2*max_count x 256, with essentially v1's DMA volume.

DRAM layouts per core (FL = F/2 = 2048, FLO = FL/128 = 16):
  x   [n_tiles, 128, KO, CT] bf16  slot-A tiles then slot-B tiles
  w1  [2, 4, 128, KO, 512]   bf16  w1[s,q,p,ko,ff] = w1_{e_s}[ko*128+p, h*FL+q*512+ff]
  w2  [2, 2, 128, 8, D]      bf16  w2[s,b,p,fi,d]  = w2_{e_s}[h*FL+(b*8+fi)*128+p, d]
  b1  [128, 2*FLO]           f32   b1[p, s*FLO+fq] = b1_{e_s}[h*FL+fq*128+p]
  y   [n_tiles, 128, KO, CT] bf16  partial (gelu(x@w1l+b1l) @ w2l)^T
(h = the core's half index within its pair.)
"""

import numpy as np
import ml_dtypes

N_CORES = 8
D = 1024
F = 4096
E = 8
KO = D // 128
FL = F // 2          # 2048 local F columns per core
FLO = FL // 128      # 16 local f-chunks
CT = 512

BF16 = ml_dtypes.bfloat16

_NC_CACHE: dict[tuple, object] = {}
LAST_RESULTS = None


def _cap_tiles(C):
    tiles = []
    off = 0
    while C - off >= CT:
        tiles.append((off, CT))
        off += CT
    if off < C:
        tiles.append((off, C - off))
    return tiles


def _build(CA, CB):
    import concourse.mybir as mybir
    from concourse import bacc
    from concourse.tile import TileContext

    fp32 = mybir.dt.float32
    bf16 = mybir.dt.bfloat16

    spec = [(0, off, tw) for off, tw in _cap_tiles(CA)] + [
        (1, off, tw) for off, tw in _cap_tiles(CB)
    ]
    n_tiles = len(spec)

    nc = bacc.Bacc(
        "TRN2", target_bir_lowering=False, debug=False, num_devices=N_CORES
    )
    x = nc.dram_tensor("x", [n_tiles, 128, KO, CT], bf16, kind="ExternalInput")
    w1 = nc.dram_tensor("w1", [2, 4, 128, KO, 512], bf16, kind="ExternalInput")
    w2 = nc.dram_tensor("w2", [2, 2, 128, 8, D], bf16, kind="ExternalInput")
    b1 = nc.dram_tensor("b1", [128, 2 * FLO], fp32, kind="ExternalInput")
    y = nc.dram_tensor("y", [n_tiles, 128, KO, CT], bf16, kind="ExternalOutput")

    with TileContext(nc) as tc:
        with (
            tc.tile_pool(name="wpool", bufs=1) as wpool,
            tc.tile_pool(name="xpool", bufs=4) as xpool,
            tc.tile_pool(name="hpool", bufs=2) as hpool,
            tc.tile_pool(name="ypool", bufs=4) as ypool,
            tc.tile_pool(name="ph", bufs=4, space="PSUM") as phpool,
            tc.tile_pool(name="py", bufs=4, space="PSUM") as pypool,
        ):
            w1_sb = wpool.tile([128, 2, 4, KO, 512], bf16)
            w2_sb = wpool.tile([128, 2, FLO, D], bf16)
            b1_sb = wpool.tile([128, 2 * FLO], fp32)

            x_first = xpool.tile([128, KO, CT], bf16, tag="x_sb")
            nc.sync.dma_start(x_first[:], x[0])
            # Slot A's w1 quarters first (PE starts after 1 MB), then its
            # w2 (mm2 needs it ~30us in), then slot B's weights.
            for q in range(4):
                nc.sync.dma_start(w1_sb[:, 0, q], w1[0, q])
            nc.sync.dma_start(b1_sb[:], b1[:])
            for b in range(2):
                nc.sync.dma_start(w2_sb[:, 0, b * 8 : (b + 1) * 8, :], w2[0, b])
            for q in range(4):
                nc.sync.dma_start(w1_sb[:, 1, q], w1[1, q])
            for b in range(2):
                nc.sync.dma_start(w2_sb[:, 1, b * 8 : (b + 1) * 8, :], w2[1, b])

            for ti, (s, off, tw) in enumerate(spec):
                if ti == 0:
                    x_sb = x_first
                else:
                    x_sb = xpool.tile([128, KO, CT], bf16, tag="x_sb")
                    nc.sync.dma_start(x_sb[:], x[ti])
                h_sb = hpool.tile([128, FLO, CT], bf16)
                for fo in range(FLO):
                    q, fq = divmod(fo, 4)
                    ph = phpool.tile([128, CT], fp32)
                    for ko in range(KO):
                        nc.tensor.matmul(
                            ph[:, :tw],
                            lhsT=w1_sb[:, s, q, ko, fq * 128 : (fq + 1) * 128],
                            rhs=x_sb[:, ko, :tw],
                            start=(ko == 0),
                            stop=(ko == KO - 1),
                        )
                    nc.scalar.activation(
                        h_sb[:, fo, :tw],
                        ph[:, :tw],
                        mybir.ActivationFunctionType.Gelu,
                        bias=b1_sb[:, s * FLO + fo : s * FLO + fo + 1],
                    )
                for do in range(KO):
                    py = pypool.tile([128, CT], fp32)
                    for fo in range(FLO):
                        nc.tensor.matmul(
                            py[:, :tw],
                            lhsT=w2_sb[:, s, fo, do * 128 : (do + 1) * 128],
                            rhs=h_sb[:, fo, :tw],
                            start=(fo == 0),
                            stop=(fo == FLO - 1),
                        )
                    y_do = ypool.tile([128, CT], bf16, tag="y_do")
                    nc.vector.tensor_copy(y_do[:, :tw], py[:, :tw])
                    # Full-width DMA: contiguous rows (128 descriptors, no
                    # strided slow path); pad columns carry ignored stale
                    # data. Per-do DMAs pipeline under the remaining mm2s,
                    # so the kernel tail only waits on one 128 KB transfer.
                    nc.sync.dma_start(y[ti][:, do, :], y_do[:])

    nc.compile()
    return nc, spec


def kernel(x, gate_w, w1, b1, w2, b2):
    from concourse.bass_utils import run_bass_kernel_spmd

    global LAST_RESULTS

    x = np.asarray(x, dtype=np.float32)
    gate_w = np.asarray(gate_w, dtype=np.float32)
    w1 = np.asarray(w1, dtype=np.float32)
    b1 = np.asarray(b1, dtype=np.float32)
    w2 = np.asarray(w2, dtype=np.float32)
    b2 = np.asarray(b2, dtype=np.float32)

    B, S, Din = x.shape
    assert Din == D and gate_w.shape == (D, E)
    T = B * S
    xf = x.reshape(T, D)

    # ---- Host router + dispatch (as v1) ----
    logits = xf.astype(np.float64) @ gate_w.astype(np.float64)
    idx0 = np.argmax(logits, axis=1)
    rows = np.arange(T)
    v0 = logits[rows, idx0]
    l2 = logits.copy()
    l2[rows, idx0] = -np.inf
    idx1 = np.argmax(l2, axis=1)
    v1_ = l2[rows, idx1]
    e1 = np.exp(v1_ - v0)
    cw0 = 1.0 / (1.0 + e1)
    cw1 = e1 / (1.0 + e1)

    token_ids = []
    combine_w = []
    for e in range(E):
        sel0 = idx0 == e
        sel1 = idx1 == e
        ids = np.nonzero(sel0 | sel1)[0]
        w = np.where(sel0[ids], cw0[ids], cw1[ids])
        token_ids.append(ids)
        combine_w.append(w)

    counts = np.array([len(ids) for ids in token_ids])
    # Pair i-th largest with i-th smallest; slot A = the large expert.
    order = np.argsort(-counts)
    pairs = [(int(order[i]), int(order[E - 1 - i])) for i in range(E // 2)]
    CA = int(max(counts[eA] for eA, _ in pairs))
    CB = int(max(counts[eB] for _, eB in pairs))
    CA += CA & 1
    CB += CB & 1

    if (CA, CB) not in _NC_CACHE:
        _NC_CACHE[(CA, CB)] = _build(CA, CB)
    nc, spec = _NC_CACHE[(CA, CB)]
    n_tiles = len(spec)

    # ---- Per-pair token tiles; per-core weight halves ----
    in_maps = [None] * N_CORES
    pair_x = []
    for pi, (eA, eB) in enumerate(pairs):
        xtiles = np.zeros((n_tiles, 128, KO, CT), dtype=BF16)
        for ti, (s, off, tw) in enumerate(spec):
            e = (eA, eB)[s]
            ids_seg = token_ids[e][off : off + tw]
            w_val = len(ids_seg)
            if w_val == 0:
                continue
            blk = (
                xf[ids_seg].astype(BF16).reshape(w_val, KO, 128).transpose(2, 1, 0)
            )
            xtiles[ti, :, :, :w_val] = blk
        xtiles = np.ascontiguousarray(xtiles)
        pair_x.append(xtiles)
        for h in range(2):
            sl = slice(h * FL, (h + 1) * FL)
            w1c = np.stack(
                [
                    w1[e][:, sl]
                    .reshape(KO, 128, 4, 512)
                    .transpose(2, 1, 0, 3)
                    for e in (eA, eB)
                ]
            ).astype(BF16)  # [2, 4, 128, KO, 512]
            w2c = np.stack(
                [
                    w2[e][sl, :]
                    .reshape(2, 8, 128, D)
                    .transpose(0, 2, 1, 3)
                    for e in (eA, eB)
                ]
            ).astype(BF16)  # [2, 2, 128, 8, D]
            b1c = np.concatenate(
                [b1[e][sl].reshape(FLO, 128).T for e in (eA, eB)], axis=1
            )  # [128, 2*FLO]
            in_maps[2 * pi + h] = {
                "x": xtiles,
                "w1": np.ascontiguousarray(w1c),
                "w2": np.ascontiguousarray(w2c),
                "b1": np.ascontiguousarray(b1c),
            }

    res = run_bass_kernel_spmd(nc, in_maps, core_ids=list(range(N_CORES)))
    LAST_RESULTS = res

    # ---- Host: sum the pair halves, combine, scatter ----
    out = np.zeros((T, D), dtype=np.float32)
    for pi, (eA, eB) in enumerate(pairs):
        ysum = res.results[2 * pi]["y"].astype(np.float32) + res.results[
            2 * pi + 1
        ]["y"].astype(np.float32)
        for ti, (s, off, tw) in enumerate(spec):
            e = (eA, eB)[s]
            ids_seg = token_ids[e][off : off + tw]
            w_val = len(ids_seg)
            if w_val == 0:
                continue
            cw_seg = combine_w[e][off : off + w_val].astype(np.float32)
            yt = ysum[ti, :, :, :w_val].transpose(2, 1, 0).reshape(w_val, D)
            out[ids_seg] += cw_seg[:, None] * (yt + b2[e])

    return out.reshape(B, S, D)



# revision 5
# speedup vs baseline: 1.0010x; 1.0010x over previous
"""MoE kernel v3: pair-wise F-split (2 cores per expert pair, F/2 each).

Experts are paired largest-with-smallest; the pair's two cores each hold
the F-half of BOTH experts (128 KB/partition, same as v1) and process all
tokens of both experts on their half. Slot capacities are global
(CA = largest expert count, CB = largest count among the 4 "small" slot
experts), so the program is SPMD; per-core data decides which experts a
core serves. Partial outputs (bf16) from the two cores of a pair are
summed on host, then combined/scattered as in v1.

Per-core PE work: (CA + CB) columns x 256 cycles — ~4% less than v1's
2*max_count x 256, with essentially v1's DMA volume.

DRAM layouts per core (FL = F/2 = 2048, FLO = FL/128 = 16):
  x   [n_tiles, 128, KO, CT] bf16  slot-A tiles then slot-B tiles
  w1  [2, 4, 128, KO, 512]   bf16  w1[s,q,p,ko,ff] = w1_{e_s}[ko*128+p, h*FL+q*512+ff]
  w2  [2, 2, 128, 8, D]      bf16  w2[s,b,p,fi,d]  = w2_{e_s}[h*FL+(b*8+fi)*128+p, d]
  b1  [128, 2*FLO]           f32   b1[p, s*FLO+fq] = b1_{e_s}[h*FL+fq*128+p]
  y   [n_tiles, 128, KO, CT] bf16  partial (gelu(x@w1l+b1l) @ w2l)^T
(h = the core's half index within its pair.)
"""

import numpy as np
import ml_dtypes

N_CORES = 8
D = 1024
F = 4096
E = 8
KO = D // 128
FL = F // 2          # 2048 local F columns per core
FLO = FL // 128      # 16 local f-chunks
CT = 512

BF16 = ml_dtypes.bfloat16

_NC_CACHE: dict[tuple, object] = {}
LAST_RESULTS = None


def _cap_tiles(C):
    tiles = []
    off = 0
    while C - off >= CT:
        tiles.append((off, CT))
        off += CT
    if off < C:
        tiles.append((off, C - off))
    return tiles


def _build(CA, CB):
    import concourse.mybir as mybir
    from concourse import bacc
    from concourse.tile import TileContext

    fp32 = mybir.dt.float32
    bf16 = mybir.dt.bfloat16

    spec = [(0, off, tw) for off, tw in _cap_tiles(CA)] + [
        (1, off, tw) for off, tw in _cap_tiles(CB)
    ]
    # Smallest tile last: shrinks the kernel tail (final mm2 group + y
    # copy/DMA drain scales with the last tile's width).
    tail = min(range(len(spec)), key=lambda i: spec[i][2])
    spec.append(spec.pop(tail))
    n_tiles = len(spec)

    nc = bacc.Bacc(
        "TRN2", target_bir_lowering=False, debug=False, num_devices=N_CORES
    )
    x = nc.dram_tensor("x", [n_tiles, 128, KO, CT], bf16, kind="ExternalInput")
    w1 = nc.dram_tensor("w1", [2, 4, 4, 128, KO, 128], bf16, kind="ExternalInput")
    w2 = nc.dram_tensor("w2", [2, 2, 128, 8, D], bf16, kind="ExternalInput")
    b1 = nc.dram_tensor("b1", [128, 2 * FLO], fp32, kind="ExternalInput")
    y = nc.dram_tensor("y", [n_tiles, 128, KO, CT], bf16, kind="ExternalOutput")

    with TileContext(nc) as tc:
        with (
            tc.tile_pool(name="wpool", bufs=1) as wpool,
            tc.tile_pool(name="xpool", bufs=4) as xpool,
            tc.tile_pool(name="hpool", bufs=2) as hpool,
            tc.tile_pool(name="ypool", bufs=4) as ypool,
            tc.tile_pool(name="ph", bufs=4, space="PSUM") as phpool,
            tc.tile_pool(name="py", bufs=4, space="PSUM") as pypool,
        ):
            w1_sb = wpool.tile([128, 2, 4, 4, KO, 128], bf16)
            w2_sb = wpool.tile([128, 2, FLO, D], bf16)
            b1_sb = wpool.tile([128, 2 * FLO], fp32)

            x_first = xpool.tile([128, KO, CT], bf16, tag="x_sb")
            # Fine-grained critical path: the first mm1 column block (fo=0)
            # needs only x[0] + w1[s0,q0,fq0] (256 KB). Split both DMAs so
            # the PE starts ~1.5 us in instead of waiting for 1 MB chunks.
            for ko in range(KO):
                nc.sync.dma_start(x_first[:, ko], x[0][:, ko])
            for fq in range(4):
                nc.sync.dma_start(w1_sb[:, 0, 0, fq], w1[0, 0, fq])
            nc.sync.dma_start(b1_sb[:], b1[:])
            for q in range(1, 4):
                for fq in range(4):
                    nc.sync.dma_start(w1_sb[:, 0, q, fq], w1[0, q, fq])
            for b in range(2):
                nc.sync.dma_start(w2_sb[:, 0, b * 8 : (b + 1) * 8, :], w2[0, b])
            for q in range(4):
                for fq in range(4):
                    nc.sync.dma_start(w1_sb[:, 1, q, fq], w1[1, q, fq])
            for b in range(2):
                nc.sync.dma_start(w2_sb[:, 1, b * 8 : (b + 1) * 8, :], w2[1, b])

            for ti, (s, off, tw) in enumerate(spec):
                if ti == 0:
                    x_sb = x_first
                else:
                    x_sb = xpool.tile([128, KO, CT], bf16, tag="x_sb")
                    nc.sync.dma_start(x_sb[:], x[ti])
                h_sb = hpool.tile([128, FLO, CT], bf16)
                for fo in range(FLO):
                    q, fq = divmod(fo, 4)
                    ph = phpool.tile([128, CT], fp32)
                    for ko in range(KO):
                        nc.tensor.matmul(
                            ph[:, :tw],
                            lhsT=w1_sb[:, s, q, fq, ko],
                            rhs=x_sb[:, ko, :tw],
                            start=(ko == 0),
                            stop=(ko == KO - 1),
                        )
                    nc.scalar.activation(
                        h_sb[:, fo, :tw],
                        ph[:, :tw],
                        mybir.ActivationFunctionType.Gelu,
                        bias=b1_sb[:, s * FLO + fo : s * FLO + fo + 1],
                    )
                for do in range(KO):
                    py = pypool.tile([128, CT], fp32)
                    for fo in range(FLO):
                        nc.tensor.matmul(
                            py[:, :tw],
                            lhsT=w2_sb[:, s, fo, do * 128 : (do + 1) * 128],
                            rhs=h_sb[:, fo, :tw],
                            start=(fo == 0),
                            stop=(fo == FLO - 1),
                        )
                    y_do = ypool.tile([128, CT], bf16, tag="y_do")
                    nc.vector.tensor_copy(y_do[:, :tw], py[:, :tw])
                    # Full-width DMA: contiguous rows (128 descriptors, no
                    # strided slow path); pad columns carry ignored stale
                    # data. Per-do DMAs pipeline under the remaining mm2s,
                    # so the kernel tail only waits on one 128 KB transfer.
                    nc.sync.dma_start(y[ti][:, do, :], y_do[:])

    nc.compile()
    return nc, spec


def kernel(x, gate_w, w1, b1, w2, b2):
    from concourse.bass_utils import run_bass_kernel_spmd

    global LAST_RESULTS

    x = np.asarray(x, dtype=np.float32)
    gate_w = np.asarray(gate_w, dtype=np.float32)
    w1 = np.asarray(w1, dtype=np.float32)
    b1 = np.asarray(b1, dtype=np.float32)
    w2 = np.asarray(w2, dtype=np.float32)
    b2 = np.asarray(b2, dtype=np.float32)

    B, S, Din = x.shape
    assert Din == D and gate_w.shape == (D, E)
    T = B * S
    xf = x.reshape(T, D)

    # ---- Host router + dispatch (as v1) ----
    logits = xf.astype(np.float64) @ gate_w.astype(np.float64)
    idx0 = np.argmax(logits, axis=1)
    rows = np.arange(T)
    v0 = logits[rows, idx0]
    l2 = logits.copy()
    l2[rows, idx0] = -np.inf
    idx1 = np.argmax(l2, axis=1)
    v1_ = l2[rows, idx1]
    e1 = np.exp(v1_ - v0)
    cw0 = 1.0 / (1.0 + e1)
    cw1 = e1 / (1.0 + e1)

    token_ids = []
    combine_w = []
    for e in range(E):
        sel0 = idx0 == e
        sel1 = idx1 == e
        ids = np.nonzero(sel0 | sel1)[0]
        w = np.where(sel0[ids], cw0[ids], cw1[ids])
        token_ids.append(ids)
        combine_w.append(w)

    counts = np.array([len(ids) for ids in token_ids])
    # Pair i-th largest with i-th smallest; slot A = the large expert.
    order = np.argsort(-counts)
    pairs = [(int(order[i]), int(order[E - 1 - i])) for i in range(E // 2)]
    CA = int(max(counts[eA] for eA, _ in pairs))
    CB = int(max(counts[eB] for _, eB in pairs))
    CA += CA & 1
    CB += CB & 1

    if (CA, CB) not in _NC_CACHE:
        _NC_CACHE[(CA, CB)] = _build(CA, CB)
    nc, spec = _NC_CACHE[(CA, CB)]
    n_tiles = len(spec)

    # ---- Per-pair token tiles; per-core weight halves ----
    in_maps = [None] * N_CORES
    pair_x = []
    for pi, (eA, eB) in enumerate(pairs):
        xtiles = np.zeros((n_tiles, 128, KO, CT), dtype=BF16)
        for ti, (s, off, tw) in enumerate(spec):
            e = (eA, eB)[s]
            ids_seg = token_ids[e][off : off + tw]
            w_val = len(ids_seg)
            if w_val == 0:
                continue
            blk = (
                xf[ids_seg].astype(BF16).reshape(w_val, KO, 128).transpose(2, 1, 0)
            )
            xtiles[ti, :, :, :w_val] = blk
        xtiles = np.ascontiguousarray(xtiles)
        pair_x.append(xtiles)
        for h in range(2):
            sl = slice(h * FL, (h + 1) * FL)
            w1c = np.stack(
                [
                    w1[e][:, sl]
                    .reshape(KO, 128, 4, 4, 128)
                    .transpose(2, 3, 1, 0, 4)
                    for e in (eA, eB)
                ]
            ).astype(BF16)  # [2, 4, 4, 128, KO, 128]
            w2c = np.stack(
                [
                    w2[e][sl, :]
                    .reshape(2, 8, 128, D)
                    .transpose(0, 2, 1, 3)
                    for e in (eA, eB)
                ]
            ).astype(BF16)  # [2, 2, 128, 8, D]
            b1c = np.concatenate(
                [b1[e][sl].reshape(FLO, 128).T for e in (eA, eB)], axis=1
            )  # [128, 2*FLO]
            in_maps[2 * pi + h] = {
                "x": xtiles,
                "w1": np.ascontiguousarray(w1c),
                "w2": np.ascontiguousarray(w2c),
                "b1": np.ascontiguousarray(b1c),
            }

    res = run_bass_kernel_spmd(nc, in_maps, core_ids=list(range(N_CORES)))
    LAST_RESULTS = res

    # ---- Host: sum the pair halves, combine, scatter ----
    out = np.zeros((T, D), dtype=np.float32)
    for pi, (eA, eB) in enumerate(pairs):
        ysum = res.results[2 * pi]["y"].astype(np.float32) + res.results[
            2 * pi + 1
        ]["y"].astype(np.float32)
        for ti, (s, off, tw) in enumerate(spec):
            e = (eA, eB)[s]
            ids_seg = token_ids[e][off : off + tw]
            w_val = len(ids_seg)
            if w_val == 0:
                continue
            cw_seg = combine_w[e][off : off + w_val].astype(np.float32)
            yt = ysum[ti, :, :, :w_val].transpose(2, 1, 0).reshape(w_val, D)
            out[ids_seg] += cw_seg[:, None] * (yt + b2[e])

    return out.reshape(B, S, D)



# revision 13
# speedup vs baseline: 1.1179x; 1.1167x over previous
"""Mixed-precision MoE kernel: bf16 + fp8-DoubleRow by combine weight.

Structure: pair-wise F-split as kernel.py (2 cores per expert pair, each
core computes an F-half of both experts for all their tokens). Per
expert, the K8 tokens with the SMALLEST top-2 softmax combine weight run
entirely in fp8e4 with perf_mode=DoubleRow (~1.9x PE throughput); the
rest run in bf16. The fp8 error (~5.4% per expert contribution) is
damped by the small combine weight, keeping total rel err ~1.7e-2.

Weight residency is phased: program order A16 -> A8 -> B16 -> B8, with
ONE bf16 weight region (64 KB/part) and ONE fp8 region (32 KB/part),
each reused across slots. Slot B's weight DMAs are emitted at the phase
boundary; the Tile framework's WAR tracking delays them until slot A's
last reader, which leaves the whole previous phase as a prefetch window.

fp8 contraction mapping (DoubleRow slot i in {0,1}, block j):
  mm1: d = (2j+i)*128 + p,  j in 0..3   (D = 1024)
  mm2: local f = (2g+i)*128 + p, g in 0..7  (FL = 2048)

DRAM per core:
  x16 [n16, 128, KO, CT]           bf16
  x8  [n8, 128, KO2, 2, CT]        fp8   (tokens * SX)
  w1_16 [2, 128, 4, 4, KO, 128]    bf16
  w2_16 [2, 128, FLO, D]           bf16
  w1_8  [2, 128, 4, 4, KO2, 2, 128] fp8  (w1 * SW1)
  w2_8  [2, 128, FLO2, 2, D]       fp8  (w2 * SW2)
  b1  [128, 2*FLO]                 f32
  y   [n_tiles, 128, KO, CT]       bf16  (fp8 tiles carry y * SW2)
"""

import numpy as np
import ml_dtypes

N_CORES = 8
D = 1024
F = 4096
E = 8
KO = D // 128
KO2 = KO // 2
FL = F // 2
FLO = FL // 128
FLO2 = FLO // 2
CT = 512

AVG_K8 = 512  # target average per-expert fp8 token count

ACT_IDENTITY = False  # debug: CoreSim has no Gelu; swap for Identity

SX = 16.0
SW1 = 256.0
SW2 = 512.0

BF16 = ml_dtypes.bfloat16
FP8 = ml_dtypes.float8_e4m3

_NC_CACHE: dict[tuple, object] = {}
LAST_RESULTS = None


def _cap_tiles(C):
    tiles = []
    off = 0
    while C - off >= CT:
        tiles.append((off, CT))
        off += CT
    if off < C:
        tiles.append((off, C - off))
    return tiles


def _eq_tiles(C):
    # Equal-width tiles: narrow matmuls (< ~260 cols) fall to the
    # LDWEIGHTS cadence (~107 ns/MM), so spread the remainder evenly
    # instead of emitting one thin tail tile.
    import math

    n = max(1, math.ceil(C / CT))
    base, extra = divmod(C, n)
    tiles = []
    off = 0
    for i in range(n):
        tw = base + (1 if i < extra else 0)
        tiles.append((off, tw))
        off += tw
    return tiles


def _build(CA16, CB16, K8A, K8B):
    import concourse.mybir as mybir
    from concourse import bacc
    from concourse.tile import TileContext

    fp32 = mybir.dt.float32
    bf16 = mybir.dt.bfloat16
    fp8 = mybir.dt.float8e4
    DR = mybir.MatmulPerfMode.DoubleRow

    # (kind, slot, off, tw): kind 0 = bf16, 1 = fp8.
    # Phase order A16 -> A8 -> B16 -> B8 (weight prefetch windows).
    specA16 = [(0, 0, off, tw) for off, tw in _eq_tiles(CA16)]
    specA8 = [(1, 0, off, tw) for off, tw in _cap_tiles(K8A)]
    specB16 = [(0, 1, off, tw) for off, tw in _eq_tiles(CB16)]
    specB8 = [(1, 1, off, tw) for off, tw in _cap_tiles(K8B)]
    spec = specA16 + specA8 + specB16 + specB8
    n16 = len(specA16) + len(specB16)
    n8 = len(specA8) + len(specB8)
    n_tiles = len(spec)

    nc = bacc.Bacc(
        "TRN2", target_bir_lowering=False, debug=False, num_devices=N_CORES
    )
    x16 = nc.dram_tensor("x16", [n16, 128, KO, CT], bf16, kind="ExternalInput")
    x8 = (
        nc.dram_tensor("x8", [n8, 128, KO2, 2, CT], fp8, kind="ExternalInput")
        if n8
        else None
    )
    w1_16 = nc.dram_tensor(
        "w1_16", [2, 128, 4, 4, KO, 128], bf16, kind="ExternalInput"
    )
    w2_16 = nc.dram_tensor("w2_16", [2, 128, FLO, D], bf16, kind="ExternalInput")
    w1_8 = (
        nc.dram_tensor("w1_8", [2, 128, 4, 4, KO2, 2, 128], fp8, kind="ExternalInput")
        if n8
        else None
    )
    w2_8 = (
        nc.dram_tensor("w2_8", [2, 128, FLO2, 2, D], fp8, kind="ExternalInput")
        if n8
        else None
    )
    b1 = nc.dram_tensor("b1", [128, 2 * FLO], fp32, kind="ExternalInput")
    y = nc.dram_tensor("y", [n_tiles, 128, KO, CT], bf16, kind="ExternalOutput")

    with TileContext(nc) as tc:
        with (
            tc.tile_pool(name="wpool", bufs=1) as wpool,
            tc.tile_pool(name="xpool", bufs=3) as xpool,
            tc.tile_pool(name="x8pool", bufs=2) as x8pool,
            tc.tile_pool(name="hpool", bufs=2) as hpool,
            tc.tile_pool(name="h8pool", bufs=2) as h8pool,
            tc.tile_pool(name="ypool", bufs=4) as ypool,
            tc.tile_pool(name="ph", bufs=4, space="PSUM") as phpool,
            tc.tile_pool(name="py", bufs=4, space="PSUM") as pypool,
        ):
            # Single-slot weight regions, reused A -> B.
            w1s = wpool.tile([128, 4, 4, KO, 128], bf16)
            w2s = wpool.tile([128, FLO, D], bf16)
            w1s8 = wpool.tile([128, 4, 4, KO2, 2, 128], fp8)
            w2s8 = wpool.tile([128, FLO2, 2, D], fp8)
            b1_sb = wpool.tile([128, 2 * FLO], fp32)

            x_first = xpool.tile([128, KO, CT], bf16, tag="x16_sb")
            # Startup: first mm1 column block needs x16[0] + w1_16 A [q0,fq0].
            nc.sync.dma_start(x_first[:, 0], x16[0][:, 0])
            nc.sync.dma_start(w1s[:, 0, 0], w1_16[0, :, 0, 0])
            nc.sync.dma_start(b1_sb[:], b1[:])
            for fq in range(1, 4):
                nc.sync.dma_start(x_first[:, fq], x16[0][:, fq])
                nc.sync.dma_start(w1s[:, 0, fq], w1_16[0, :, 0, fq])
            for ko in range(4, KO):
                nc.sync.dma_start(x_first[:, ko], x16[0][:, ko])
                nc.sync.dma_start(w1s[:, 1, ko - 4], w1_16[0, :, 1, ko - 4])
            for q in range(2, 4):
                nc.sync.dma_start(w1s[:, q], w1_16[0, :, q])
            for b in range(2):
                nc.sync.dma_start(
                    w2s[:, b * 8 : (b + 1) * 8], w2_16[0, :, b * 8 : (b + 1) * 8]
                )
            # Slot A fp8 weights (used in phase A8, prefetched under A16).
            if n8:
                nc.sync.dma_start(w1s8[:], w1_8[0])
                nc.sync.dma_start(w2s8[:], w2_8[0])

            i16 = 0
            i8 = 0
            for ti, (kind, s, off, tw) in enumerate(spec):
                if kind == 0 and s == 1 and off == 0:
                    # Entering phase B16: slot B bf16 weights into the shared
                    # region. WAR deps on A16's matmuls order these after the
                    # last A16 reader; they stream during phase A8.
                    for q in range(4):
                        nc.sync.dma_start(w1s[:, q], w1_16[1, :, q])
                    for b in range(2):
                        nc.sync.dma_start(
                            w2s[:, b * 8 : (b + 1) * 8],
                            w2_16[1, :, b * 8 : (b + 1) * 8],
                        )
                if kind == 1 and s == 1 and off == 0:
                    # Entering phase B8: slot B fp8 weights (prefetch under
                    # B16, ordered after A8's readers).
                    nc.sync.dma_start(w1s8[:], w1_8[1])
                    nc.sync.dma_start(w2s8[:], w2_8[1])

                if kind == 0:
                    if i16 == 0:
                        x_sb = x_first
                    else:
                        x_sb = xpool.tile([128, KO, CT], bf16, tag="x16_sb")
                        nc.sync.dma_start(x_sb[:], x16[i16])
                    i16 += 1
                    h_sb = hpool.tile([128, FLO, CT], bf16)
                    for fo in range(FLO):
                        q, fq = divmod(fo, 4)
                        ph = phpool.tile([128, CT], fp32)
                        for ko in range(KO):
                            nc.tensor.matmul(
                                ph[:, :tw],
                                lhsT=w1s[:, q, fq, ko],
                                rhs=x_sb[:, ko, :tw],
                                start=(ko == 0),
                                stop=(ko == KO - 1),
                            )
                        nc.scalar.activation(
                            h_sb[:, fo, :tw],
                            ph[:, :tw],
                            (mybir.ActivationFunctionType.Identity if ACT_IDENTITY else mybir.ActivationFunctionType.Gelu),
                            bias=b1_sb[:, s * FLO + fo : s * FLO + fo + 1],
                        )
                    for do in range(KO):
                        py = pypool.tile([128, CT], fp32)
                        for fo in range(FLO):
                            nc.tensor.matmul(
                                py[:, :tw],
                                lhsT=w2s[:, fo, do * 128 : (do + 1) * 128],
                                rhs=h_sb[:, fo, :tw],
                                start=(fo == 0),
                                stop=(fo == FLO - 1),
                            )
                        y_do = ypool.tile([128, CT], bf16, tag="y_do")
                        nc.vector.tensor_copy(y_do[:, :tw], py[:, :tw])
                        nc.sync.dma_start(y[ti][:, do, :tw], y_do[:, :tw])
                else:
                    x_sb = x8pool.tile([128, KO2, 2, CT], fp8, tag="x8_sb")
                    nc.sync.dma_start(x_sb[:], x8[i8])
                    i8 += 1
                    h_sb = h8pool.tile([128, FLO2, 2, CT], fp8)
                    for fo in range(FLO):
                        q, fq = divmod(fo, 4)
                        ph = phpool.tile([128, CT], fp32)
                        for j in range(KO2):
                            nc.tensor.matmul(
                                ph[:, :tw],
                                lhsT=w1s8[:, q, fq, j],
                                rhs=x_sb[:, j, :, :tw],
                                start=(j == 0),
                                stop=(j == KO2 - 1),
                                perf_mode=DR,
                            )
                        nc.scalar.activation(
                            h_sb[:, fo // 2, fo % 2, :tw],
                            ph[:, :tw],
                            (mybir.ActivationFunctionType.Identity if ACT_IDENTITY else mybir.ActivationFunctionType.Gelu),
                            bias=b1_sb[:, s * FLO + fo : s * FLO + fo + 1],
                            scale=1.0 / (SX * SW1),
                        )
                    for do in range(KO):
                        py = pypool.tile([128, CT], fp32)
                        for g in range(FLO2):
                            nc.tensor.matmul(
                                py[:, :tw],
                                lhsT=w2s8[:, g, :, do * 128 : (do + 1) * 128],
                                rhs=h_sb[:, g, :, :tw],
                                start=(g == 0),
                                stop=(g == FLO2 - 1),
                                perf_mode=DR,
                            )
                        y_do = ypool.tile([128, CT], bf16, tag="y_do")
                        nc.vector.tensor_copy(y_do[:, :tw], py[:, :tw])
                        nc.sync.dma_start(y[ti][:, do, :tw], y_do[:, :tw])

    nc.compile()
    return nc, spec


def kernel(x, gate_w, w1, b1, w2, b2):
    from concourse.bass_utils import run_bass_kernel_spmd

    global LAST_RESULTS

    x = np.asarray(x, dtype=np.float32)
    gate_w = np.asarray(gate_w, dtype=np.float32)
    w1 = np.asarray(w1, dtype=np.float32)
    b1 = np.asarray(b1, dtype=np.float32)
    w2 = np.asarray(w2, dtype=np.float32)
    b2 = np.asarray(b2, dtype=np.float32)

    B, S, Din = x.shape
    T = B * S
    xf = x.reshape(T, D)

    # ---- Host router ----
    logits = xf.astype(np.float64) @ gate_w.astype(np.float64)
    idx0 = np.argmax(logits, axis=1)
    rows = np.arange(T)
    v0 = logits[rows, idx0]
    l2 = logits.copy()
    l2[rows, idx0] = -np.inf
    idx1 = np.argmax(l2, axis=1)
    v1_ = l2[rows, idx1]
    e1 = np.exp(v1_ - v0)
    cw0 = 1.0 / (1.0 + e1)
    cw1 = e1 / (1.0 + e1)

    # Per expert: tokens sorted by combine weight ascending; the first K8
    # go to the fp8 path.
    token_ids = []
    combine_w = []
    for e in range(E):
        sel0 = idx0 == e
        sel1 = idx1 == e
        ids = np.nonzero(sel0 | sel1)[0]
        w = np.where(sel0[ids], cw0[ids], cw1[ids])
        o = np.argsort(w)
        token_ids.append(ids[o])
        combine_w.append(w[o])

    counts = np.array([len(ids) for ids in token_ids])
    order = np.argsort(-counts)
    pairs = [(int(order[i]), int(order[E - 1 - i])) for i in range(E // 2)]
    CA = int(max(counts[eA] for eA, _ in pairs))
    CB = int(max(counts[eB] for _, eB in pairs))
    # Uniform fp8 share: DR tiles must be full 512 wide (narrower DR
    # matmuls run at the 213 ns LDWEIGHTS cadence), so K8 = 512 exactly.
    K8A = K8B = AVG_K8
    CA16 = CA - K8A
    CB16 = CB - K8B
    CA16 += CA16 & 1
    CB16 += CB16 & 1
    # per-expert fp8 counts
    K8e = np.zeros(E, dtype=int)
    for a, b in pairs:
        K8e[a] = counts[a] - CA16
        K8e[b] = counts[b] - CB16

    key = (CA16, CB16, K8A, K8B)
    if key not in _NC_CACHE:
        _NC_CACHE[key] = _build(CA16, CB16, K8A, K8B)
    nc, spec = _NC_CACHE[key]
    n_tiles = len(spec)

    xq = np.asarray(xf * SX, dtype=FP8)

    in_maps = [None] * N_CORES
    for pi, (eA, eB) in enumerate(pairs):
        n16 = sum(1 for k, _, _, _ in spec if k == 0)
        n8 = n_tiles - n16
        x16t = np.zeros((n16, 128, KO, CT), dtype=BF16)
        x8t = np.zeros((n8, 128, KO2, 2, CT), dtype=FP8)
        i16 = i8 = 0
        for ti, (kind, s, off, tw) in enumerate(spec):
            e = (eA, eB)[s]
            # bf16 tokens start at K8e[e] in the sorted order; fp8 at 0
            # (clipped to the expert's actual fp8 count).
            if kind == 0:
                lo, hi = K8e[e] + off, K8e[e] + off + tw
            else:
                lo, hi = off, min(off + tw, K8e[e])
            ids_seg = token_ids[e][lo:hi]
            w_val = len(ids_seg)
            if kind == 0:
                if w_val:
                    blk = (
                        xf[ids_seg]
                        .astype(BF16)
                        .reshape(w_val, KO, 128)
                        .transpose(2, 1, 0)
                    )
                    x16t[i16, :, :, :w_val] = blk
                i16 += 1
            else:
                if w_val:
                    blk = (
                        xq[ids_seg].reshape(w_val, KO2, 2, 128).transpose(3, 1, 2, 0)
                    )
                    x8t[i8, :, :, :, :w_val] = blk
                i8 += 1
        n8_nonzero = n8 > 0
        for h in range(2):
            sl = slice(h * FL, (h + 1) * FL)
            w1c16 = np.stack(
                [
                    w1[e][:, sl]
                    .reshape(KO, 128, 4, 4, 128)
                    .transpose(1, 2, 3, 0, 4)
                    for e in (eA, eB)
                ]
            ).astype(BF16)
            w2c16 = np.stack(
                [
                    w2[e][sl, :].reshape(FLO, 128, D).transpose(1, 0, 2)
                    for e in (eA, eB)
                ]
            ).astype(BF16)
            w1c8 = np.stack(
                [
                    np.asarray(w1[e][:, sl] * SW1, dtype=FP8)
                    .reshape(KO2, 2, 128, 4, 4, 128)
                    .transpose(2, 3, 4, 0, 1, 5)
                    for e in (eA, eB)
                ]
            )
            w2c8 = np.stack(
                [
                    np.asarray(w2[e][sl, :] * SW2, dtype=FP8)
                    .reshape(FLO2, 2, 128, D)
                    .transpose(2, 0, 1, 3)
                    for e in (eA, eB)
                ]
            )
            b1c = np.concatenate(
                [b1[e][sl].reshape(FLO, 128).T for e in (eA, eB)], axis=1
            )
            im = {
                "x16": np.ascontiguousarray(x16t),
                "w1_16": np.ascontiguousarray(w1c16),
                "w2_16": np.ascontiguousarray(w2c16),
                "b1": np.ascontiguousarray(b1c),
            }
            if n8:
                im["x8"] = np.ascontiguousarray(x8t)
                im["w1_8"] = np.ascontiguousarray(w1c8)
                im["w2_8"] = np.ascontiguousarray(w2c8)
            in_maps[2 * pi + h] = im

    res = run_bass_kernel_spmd(nc, in_maps, core_ids=list(range(N_CORES)))
    LAST_RESULTS = res

    out = np.zeros((T, D), dtype=np.float32)
    for pi, (eA, eB) in enumerate(pairs):
        ysum = res.results[2 * pi]["y"].astype(np.float32) + res.results[
            2 * pi + 1
        ]["y"].astype(np.float32)
        for ti, (kind, s, off, tw) in enumerate(spec):
            e = (eA, eB)[s]
            if kind == 0:
                lo, hi = K8e[e] + off, K8e[e] + off + tw
            else:
                lo, hi = off, min(off + tw, K8e[e])
            ids_seg = token_ids[e][lo:hi]
            w_val = len(ids_seg)
            if w_val == 0:
                continue
            cw_seg = combine_w[e][lo : lo + w_val].astype(np.float32)
            yt = ysum[ti, :, :, :w_val].transpose(2, 1, 0).reshape(w_val, D)
            if kind == 1:
                yt = yt / SW2
            out[ids_seg] += cw_seg[:, None] * (yt + b2[e])

    return out.reshape(B, S, D)


# revision 15
# speedup vs baseline: 1.1324x; 1.0130x over previous
"""Mixed-precision MoE kernel: bf16 + fp8-DoubleRow by combine weight.

Structure: pair-wise F-split as kernel.py (2 cores per expert pair, each
core computes an F-half of both experts for all their tokens). Per
expert, the K8 tokens with the SMALLEST top-2 softmax combine weight run
entirely in fp8e4 with perf_mode=DoubleRow (~1.9x PE throughput); the
rest run in bf16. The fp8 error (~5.4% per expert contribution) is
damped by the small combine weight, keeping total rel err ~1.7e-2.

Weight residency is phased: program order A16 -> A8 -> B16 -> B8, with
ONE bf16 weight region (64 KB/part) and ONE fp8 region (32 KB/part),
each reused across slots. Slot B's weight DMAs are emitted at the phase
boundary; the Tile framework's WAR tracking delays them until slot A's
last reader, which leaves the whole previous phase as a prefetch window.

fp8 contraction mapping (DoubleRow slot i in {0,1}, block j):
  mm1: d = (2j+i)*128 + p,  j in 0..3   (D = 1024)
  mm2: local f = (2g+i)*128 + p, g in 0..7  (FL = 2048)

DRAM per core:
  x16 [n16, 128, KO, CT]           bf16
  x8  [n8, 128, KO2, 2, CT]        fp8   (tokens * SX)
  w1_16 [2, 128, 4, 4, KO, 128]    bf16
  w2_16 [2, 128, FLO, D]           bf16
  w1_8  [2, 128, 4, 4, KO2, 2, 128] fp8  (w1 * SW1)
  w2_8  [2, 128, FLO2, 2, D]       fp8  (w2 * SW2)
  b1  [128, 2*FLO]                 f32
  y   [n_tiles, 128, KO, CT]       bf16  (fp8 tiles carry y * SW2)
"""

import numpy as np
import ml_dtypes

N_CORES = 8
D = 1024
F = 4096
E = 8
KO = D // 128
KO2 = KO // 2
FL = F // 2
FLO = FL // 128
FLO2 = FLO // 2
CT = 512

AVG_K8 = 512  # target average per-expert fp8 token count

ACT_IDENTITY = False  # debug: CoreSim has no Gelu; swap for Identity

SX = 16.0
SW1 = 256.0
SW2 = 512.0

BF16 = ml_dtypes.bfloat16
FP8 = ml_dtypes.float8_e4m3

_NC_CACHE: dict[tuple, object] = {}
LAST_RESULTS = None


def _cap_tiles(C):
    tiles = []
    off = 0
    while C - off >= CT:
        tiles.append((off, CT))
        off += CT
    if off < C:
        tiles.append((off, C - off))
    return tiles


def _eq_tiles(C):
    # Equal-width tiles: narrow matmuls (< ~260 cols) fall to the
    # LDWEIGHTS cadence (~107 ns/MM), so spread the remainder evenly
    # instead of emitting one thin tail tile.
    import math

    n = max(1, math.ceil(C / CT))
    base, extra = divmod(C, n)
    tiles = []
    off = 0
    for i in range(n):
        tw = base + (1 if i < extra else 0)
        tiles.append((off, tw))
        off += tw
    return tiles


def _build(CA16, CB16, K8A, K8B):
    import concourse.mybir as mybir
    from concourse import bacc
    from concourse.tile import TileContext

    fp32 = mybir.dt.float32
    bf16 = mybir.dt.bfloat16
    fp8 = mybir.dt.float8e4
    DR = mybir.MatmulPerfMode.DoubleRow

    # (kind, slot, off, tw): kind 0 = bf16, 1 = fp8.
    # Phase order A16 -> A8 -> B16 -> B8 (weight prefetch windows).
    specA16 = [(0, 0, off, tw) for off, tw in _eq_tiles(CA16)]
    specA8 = [(1, 0, off, tw) for off, tw in _cap_tiles(K8A)]
    specB16 = [(0, 1, off, tw) for off, tw in _eq_tiles(CB16)]
    specB8 = [(1, 1, off, tw) for off, tw in _cap_tiles(K8B)]
    spec = specA16 + specA8 + specB16 + specB8
    n16 = len(specA16) + len(specB16)
    n8 = len(specA8) + len(specB8)
    n_tiles = len(spec)

    nc = bacc.Bacc(
        "TRN2", target_bir_lowering=False, debug=False, num_devices=N_CORES
    )
    x16 = nc.dram_tensor("x16", [n16, 128, KO, CT], bf16, kind="ExternalInput")
    x8 = (
        nc.dram_tensor("x8", [n8, 128, KO2, 2, CT], fp8, kind="ExternalInput")
        if n8
        else None
    )
    w1_16 = nc.dram_tensor(
        "w1_16", [2, 128, 4, 4, KO, 128], bf16, kind="ExternalInput"
    )
    w2_16 = nc.dram_tensor("w2_16", [2, 128, FLO, D], bf16, kind="ExternalInput")
    w1_8 = (
        nc.dram_tensor("w1_8", [2, 128, 4, 4, KO2, 2, 128], fp8, kind="ExternalInput")
        if n8
        else None
    )
    w2_8 = (
        nc.dram_tensor("w2_8", [2, 128, FLO2, 2, D], fp8, kind="ExternalInput")
        if n8
        else None
    )
    b1 = nc.dram_tensor("b1", [128, 2 * FLO], fp32, kind="ExternalInput")
    y = nc.dram_tensor("y", [n_tiles, 128, KO, CT], bf16, kind="ExternalOutput")

    with TileContext(nc) as tc:
        with (
            tc.tile_pool(name="wpool", bufs=1) as wpool,
            tc.tile_pool(name="xpool", bufs=3) as xpool,
            tc.tile_pool(name="x8pool", bufs=2) as x8pool,
            tc.tile_pool(name="hpool", bufs=2) as hpool,
            tc.tile_pool(name="h8pool", bufs=2) as h8pool,
            tc.tile_pool(name="ypool", bufs=4) as ypool,
            tc.tile_pool(name="ph", bufs=4, space="PSUM") as phpool,
            tc.tile_pool(name="py", bufs=4, space="PSUM") as pypool,
        ):
            # Single-slot weight regions, reused A -> B.
            w1s = wpool.tile([128, 4, 4, KO, 128], bf16)
            w2s = wpool.tile([128, FLO, D], bf16)
            w1s8 = wpool.tile([128, 4, 4, KO2, 2, 128], fp8)
            w2s8 = wpool.tile([128, FLO2, 2, D], fp8)
            b1_sb = wpool.tile([128, 2 * FLO], fp32)

            x_first = xpool.tile([128, KO, CT], bf16, tag="x16_sb")
            # Startup: first mm1 column block needs x16[0] + w1_16 A [q0,fq0].
            nc.sync.dma_start(x_first[:, 0], x16[0][:, 0])
            nc.sync.dma_start(w1s[:, 0, 0], w1_16[0, :, 0, 0])
            nc.sync.dma_start(b1_sb[:], b1[:])
            for fq in range(1, 4):
                nc.sync.dma_start(x_first[:, fq], x16[0][:, fq])
                nc.sync.dma_start(w1s[:, 0, fq], w1_16[0, :, 0, fq])
            for ko in range(4, KO):
                nc.sync.dma_start(x_first[:, ko], x16[0][:, ko])
                nc.sync.dma_start(w1s[:, 1, ko - 4], w1_16[0, :, 1, ko - 4])
            for q in range(2, 4):
                nc.sync.dma_start(w1s[:, q], w1_16[0, :, q])
            for b in range(4):
                nc.sync.dma_start(
                    w2s[:, b * 4 : (b + 1) * 4], w2_16[0, :, b * 4 : (b + 1) * 4]
                )
            # Slot A fp8 weights (used in phase A8, prefetched under A16).
            if n8:
                nc.sync.dma_start(w1s8[:], w1_8[0])
                nc.sync.dma_start(w2s8[:], w2_8[0])

            i16 = 0
            i8 = 0
            for ti, (kind, s, off, tw) in enumerate(spec):
                if kind == 0 and s == 1 and off == 0:
                    # Entering phase B16: slot B bf16 weights into the shared
                    # region. WAR deps on A16's matmuls order these after the
                    # last A16 reader; they stream during phase A8.
                    for q in range(4):
                        nc.sync.dma_start(w1s[:, q], w1_16[1, :, q])
                    for b in range(2):
                        nc.sync.dma_start(
                            w2s[:, b * 8 : (b + 1) * 8],
                            w2_16[1, :, b * 8 : (b + 1) * 8],
                        )
                if kind == 1 and s == 1 and off == 0:
                    # Entering phase B8: slot B fp8 weights (prefetch under
                    # B16, ordered after A8's readers).
                    nc.sync.dma_start(w1s8[:], w1_8[1])
                    nc.sync.dma_start(w2s8[:], w2_8[1])

                if kind == 0:
                    if i16 == 0:
                        x_sb = x_first
                    else:
                        x_sb = xpool.tile([128, KO, CT], bf16, tag="x16_sb")
                        nc.sync.dma_start(x_sb[:], x16[i16])
                    i16 += 1
                    h_sb = hpool.tile([128, FLO, CT], bf16)
                    for fo in range(FLO):
                        q, fq = divmod(fo, 4)
                        ph = phpool.tile([128, CT], fp32)
                        for ko in range(KO):
                            nc.tensor.matmul(
                                ph[:, :tw],
                                lhsT=w1s[:, q, fq, ko],
                                rhs=x_sb[:, ko, :tw],
                                start=(ko == 0),
                                stop=(ko == KO - 1),
                            )
                        nc.scalar.activation(
                            h_sb[:, fo, :tw],
                            ph[:, :tw],
                            (mybir.ActivationFunctionType.Identity if ACT_IDENTITY else mybir.ActivationFunctionType.Gelu),
                            bias=b1_sb[:, s * FLO + fo : s * FLO + fo + 1],
                        )
                    for do in range(KO):
                        py = pypool.tile([128, CT], fp32)
                        for fo in range(FLO):
                            nc.tensor.matmul(
                                py[:, :tw],
                                lhsT=w2s[:, fo, do * 128 : (do + 1) * 128],
                                rhs=h_sb[:, fo, :tw],
                                start=(fo == 0),
                                stop=(fo == FLO - 1),
                            )
                        y_do = ypool.tile([128, CT], bf16, tag="y_do")
                        nc.vector.tensor_copy(y_do[:, :tw], py[:, :tw])
                        nc.sync.dma_start(y[ti][:, do, :tw], y_do[:, :tw])
                else:
                    x_sb = x8pool.tile([128, KO2, 2, CT], fp8, tag="x8_sb")
                    nc.sync.dma_start(x_sb[:], x8[i8])
                    i8 += 1
                    h_sb = h8pool.tile([128, FLO2, 2, CT], fp8)
                    for fo in range(FLO):
                        q, fq = divmod(fo, 4)
                        ph = phpool.tile([128, CT], fp32)
                        for j in range(KO2):
                            nc.tensor.matmul(
                                ph[:, :tw],
                                lhsT=w1s8[:, q, fq, j],
                                rhs=x_sb[:, j, :, :tw],
                                start=(j == 0),
                                stop=(j == KO2 - 1),
                                perf_mode=DR,
                            )
                        nc.scalar.activation(
                            h_sb[:, fo // 2, fo % 2, :tw],
                            ph[:, :tw],
                            (mybir.ActivationFunctionType.Identity if ACT_IDENTITY else mybir.ActivationFunctionType.Gelu),
                            bias=b1_sb[:, s * FLO + fo : s * FLO + fo + 1],
                            scale=1.0 / (SX * SW1),
                        )
                    for do in range(KO):
                        py = pypool.tile([128, CT], fp32)
                        for g in range(FLO2):
                            nc.tensor.matmul(
                                py[:, :tw],
                                lhsT=w2s8[:, g, :, do * 128 : (do + 1) * 128],
                                rhs=h_sb[:, g, :, :tw],
                                start=(g == 0),
                                stop=(g == FLO2 - 1),
                                perf_mode=DR,
                            )
                        y_do = ypool.tile([128, CT], bf16, tag="y_do")
                        nc.vector.tensor_copy(y_do[:, :tw], py[:, :tw])
                        nc.sync.dma_start(y[ti][:, do, :tw], y_do[:, :tw])

    nc.compile()
    return nc, spec


def kernel(x, gate_w, w1, b1, w2, b2):
    from concourse.bass_utils import run_bass_kernel_spmd

    global LAST_RESULTS

    x = np.asarray(x, dtype=np.float32)
    gate_w = np.asarray(gate_w, dtype=np.float32)
    w1 = np.asarray(w1, dtype=np.float32)
    b1 = np.asarray(b1, dtype=np.float32)
    w2 = np.asarray(w2, dtype=np.float32)
    b2 = np.asarray(b2, dtype=np.float32)

    B, S, Din = x.shape
    T = B * S
    xf = x.reshape(T, D)

    # ---- Host router ----
    logits = xf.astype(np.float64) @ gate_w.astype(np.float64)
    idx0 = np.argmax(logits, axis=1)
    rows = np.arange(T)
    v0 = logits[rows, idx0]
    l2 = logits.copy()
    l2[rows, idx0] = -np.inf
    idx1 = np.argmax(l2, axis=1)
    v1_ = l2[rows, idx1]
    e1 = np.exp(v1_ - v0)
    cw0 = 1.0 / (1.0 + e1)
    cw1 = e1 / (1.0 + e1)

    # Per expert: tokens sorted by combine weight ascending; the first K8
    # go to the fp8 path.
    token_ids = []
    combine_w = []
    for e in range(E):
        sel0 = idx0 == e
        sel1 = idx1 == e
        ids = np.nonzero(sel0 | sel1)[0]
        w = np.where(sel0[ids], cw0[ids], cw1[ids])
        o = np.argsort(w)
        token_ids.append(ids[o])
        combine_w.append(w[o])

    counts = np.array([len(ids) for ids in token_ids])
    order = np.argsort(-counts)
    pairs = [(int(order[i]), int(order[E - 1 - i])) for i in range(E // 2)]
    CA = int(max(counts[eA] for eA, _ in pairs))
    CB = int(max(counts[eB] for _, eB in pairs))
    # Uniform fp8 share: DR tiles must be full 512 wide (narrower DR
    # matmuls run at the 213 ns LDWEIGHTS cadence), so K8 = 512 exactly.
    K8A = K8B = AVG_K8
    CA16 = CA - K8A
    CB16 = CB - K8B
    CA16 += CA16 & 1
    CB16 += CB16 & 1
    # per-expert fp8 counts
    K8e = np.zeros(E, dtype=int)
    for a, b in pairs:
        K8e[a] = min(max(counts[a] - CA16, 0), K8A, counts[a])
        K8e[b] = min(max(counts[b] - CB16, 0), K8B, counts[b])

    key = (CA16, CB16, K8A, K8B)
    if key not in _NC_CACHE:
        _NC_CACHE[key] = _build(CA16, CB16, K8A, K8B)
    nc, spec = _NC_CACHE[key]
    n_tiles = len(spec)

    xq = np.asarray(xf * SX, dtype=FP8)

    in_maps = [None] * N_CORES
    for pi, (eA, eB) in enumerate(pairs):
        n16 = sum(1 for k, _, _, _ in spec if k == 0)
        n8 = n_tiles - n16
        x16t = np.zeros((n16, 128, KO, CT), dtype=BF16)
        x8t = np.zeros((n8, 128, KO2, 2, CT), dtype=FP8)
        i16 = i8 = 0
        for ti, (kind, s, off, tw) in enumerate(spec):
            e = (eA, eB)[s]
            # bf16 tokens start at K8e[e] in the sorted order; fp8 at 0
            # (clipped to the expert's actual fp8 count).
            if kind == 0:
                lo, hi = K8e[e] + off, K8e[e] + off + tw
            else:
                lo, hi = off, min(off + tw, K8e[e])
            ids_seg = token_ids[e][lo:hi]
            w_val = len(ids_seg)
            if kind == 0:
                if w_val:
                    blk = (
                        xf[ids_seg]
                        .astype(BF16)
                        .reshape(w_val, KO, 128)
                        .transpose(2, 1, 0)
                    )
                    x16t[i16, :, :, :w_val] = blk
                i16 += 1
            else:
                if w_val:
                    blk = (
                        xq[ids_seg].reshape(w_val, KO2, 2, 128).transpose(3, 1, 2, 0)
                    )
                    x8t[i8, :, :, :, :w_val] = blk
                i8 += 1
        n8_nonzero = n8 > 0
        for h in range(2):
            sl = slice(h * FL, (h + 1) * FL)
            w1c16 = np.stack(
                [
                    w1[e][:, sl]
                    .reshape(KO, 128, 4, 4, 128)
                    .transpose(1, 2, 3, 0, 4)
                    for e in (eA, eB)
                ]
            ).astype(BF16)
            w2c16 = np.stack(
                [
                    w2[e][sl, :].reshape(FLO, 128, D).transpose(1, 0, 2)
                    for e in (eA, eB)
                ]
            ).astype(BF16)
            w1c8 = np.stack(
                [
                    np.asarray(w1[e][:, sl] * SW1, dtype=FP8)
                    .reshape(KO2, 2, 128, 4, 4, 128)
                    .transpose(2, 3, 4, 0, 1, 5)
                    for e in (eA, eB)
                ]
            )
            w2c8 = np.stack(
                [
                    np.asarray(w2[e][sl, :] * SW2, dtype=FP8)
                    .reshape(FLO2, 2, 128, D)
                    .transpose(2, 0, 1, 3)
                    for e in (eA, eB)
                ]
            )
            b1c = np.concatenate(
                [b1[e][sl].reshape(FLO, 128).T for e in (eA, eB)], axis=1
            )
            im = {
                "x16": np.ascontiguousarray(x16t),
                "w1_16": np.ascontiguousarray(w1c16),
                "w2_16": np.ascontiguousarray(w2c16),
                "b1": np.ascontiguousarray(b1c),
            }
            if n8:
                im["x8"] = np.ascontiguousarray(x8t)
                im["w1_8"] = np.ascontiguousarray(w1c8)
                im["w2_8"] = np.ascontiguousarray(w2c8)
            in_maps[2 * pi + h] = im

    res = run_bass_kernel_spmd(nc, in_maps, core_ids=list(range(N_CORES)))
    LAST_RESULTS = res

    out = np.zeros((T, D), dtype=np.float32)
    for pi, (eA, eB) in enumerate(pairs):
        ysum = res.results[2 * pi]["y"].astype(np.float32) + res.results[
            2 * pi + 1
        ]["y"].astype(np.float32)
        for ti, (kind, s, off, tw) in enumerate(spec):
            e = (eA, eB)[s]
            if kind == 0:
                lo, hi = K8e[e] + off, K8e[e] + off + tw
            else:
                lo, hi = off, min(off + tw, K8e[e])
            ids_seg = token_ids[e][lo:hi]
            w_val = len(ids_seg)
            if w_val == 0:
                continue
            cw_seg = combine_w[e][lo : lo + w_val].astype(np.float32)
            yt = ysum[ti, :, :, :w_val].transpose(2, 1, 0).reshape(w_val, D)
            if kind == 1:
                yt = yt / SW2
            out[ids_seg] += cw_seg[:, None] * (yt + b2[e])

    return out.reshape(B, S, D)


# revision 17
# speedup vs baseline: 1.1333x; 1.0008x over previous
"""Mixed-precision MoE kernel: bf16 + fp8-DoubleRow by combine weight.

Structure: pair-wise F-split as kernel.py (2 cores per expert pair, each
core computes an F-half of both experts for all their tokens). Per
expert, the K8 tokens with the SMALLEST top-2 softmax combine weight run
entirely in fp8e4 with perf_mode=DoubleRow (~1.9x PE throughput); the
rest run in bf16. The fp8 error (~5.4% per expert contribution) is
damped by the small combine weight, keeping total rel err ~1.7e-2.

Weight residency is phased: program order A16 -> A8 -> B16 -> B8, with
ONE bf16 weight region (64 KB/part) and ONE fp8 region (32 KB/part),
each reused across slots. Slot B's weight DMAs are emitted at the phase
boundary; the Tile framework's WAR tracking delays them until slot A's
last reader, which leaves the whole previous phase as a prefetch window.

fp8 contraction mapping (DoubleRow slot i in {0,1}, block j):
  mm1: d = (2j+i)*128 + p,  j in 0..3   (D = 1024)
  mm2: local f = (2g+i)*128 + p, g in 0..7  (FL = 2048)

DRAM per core:
  x16 [n16, 128, KO, CT]           bf16
  x8  [n8, 128, KO2, 2, CT]        fp8   (tokens * SX)
  w1_16 [2, 128, 4, 4, KO, 128]    bf16
  w2_16 [2, 128, FLO, D]           bf16
  w1_8  [2, 128, 4, 4, KO2, 2, 128] fp8  (w1 * SW1)
  w2_8  [2, 128, FLO2, 2, D]       fp8  (w2 * SW2)
  b1  [128, 2*FLO]                 f32
  y   [n_tiles, 128, KO, CT]       bf16  (fp8 tiles carry y * SW2)
"""

import numpy as np
import ml_dtypes

N_CORES = 8
D = 1024
F = 4096
E = 8
KO = D // 128
KO2 = KO // 2
FL = F // 2
FLO = FL // 128
FLO2 = FLO // 2
CT = 512

AVG_K8 = 512  # target average per-expert fp8 token count

ACT_IDENTITY = False  # debug: CoreSim has no Gelu; swap for Identity

SX = 16.0
SW1 = 256.0
SW2 = 512.0

BF16 = ml_dtypes.bfloat16
FP8 = ml_dtypes.float8_e4m3

_NC_CACHE: dict[tuple, object] = {}
LAST_RESULTS = None


def _cap_tiles(C):
    tiles = []
    off = 0
    while C - off >= CT:
        tiles.append((off, CT))
        off += CT
    if off < C:
        tiles.append((off, C - off))
    return tiles


def _eq_tiles(C):
    # Equal-width tiles: narrow matmuls (< ~260 cols) fall to the
    # LDWEIGHTS cadence (~107 ns/MM), so spread the remainder evenly
    # instead of emitting one thin tail tile.
    import math

    n = max(1, math.ceil(C / CT))
    base, extra = divmod(C, n)
    tiles = []
    off = 0
    for i in range(n):
        tw = base + (1 if i < extra else 0)
        tiles.append((off, tw))
        off += tw
    return tiles


def _build(CA16, CB16, K8A, K8B):
    import concourse.mybir as mybir
    from concourse import bacc
    from concourse.tile import TileContext

    fp32 = mybir.dt.float32
    bf16 = mybir.dt.bfloat16
    fp8 = mybir.dt.float8e4
    DR = mybir.MatmulPerfMode.DoubleRow

    # (kind, slot, off, tw): kind 0 = bf16, 1 = fp8.
    # Phase order A16 -> A8 -> B16 -> B8 (weight prefetch windows).
    specA16 = [(0, 0, off, tw) for off, tw in _eq_tiles(CA16)]
    specA8 = [(1, 0, off, tw) for off, tw in _cap_tiles(K8A)]
    specB16 = [(0, 1, off, tw) for off, tw in _eq_tiles(CB16)]
    specB8 = [(1, 1, off, tw) for off, tw in _cap_tiles(K8B)]
    spec = specA16 + specA8 + specB16 + specB8
    n16 = len(specA16) + len(specB16)
    n8 = len(specA8) + len(specB8)
    n_tiles = len(spec)

    nc = bacc.Bacc(
        "TRN2", target_bir_lowering=False, debug=False, num_devices=N_CORES
    )
    x16 = nc.dram_tensor("x16", [n16, 128, KO, CT], bf16, kind="ExternalInput")
    x8 = (
        nc.dram_tensor("x8", [n8, 128, KO2, 2, CT], fp8, kind="ExternalInput")
        if n8
        else None
    )
    w1_16 = nc.dram_tensor(
        "w1_16", [2, 128, 4, 4, KO, 128], bf16, kind="ExternalInput"
    )
    w2_16 = nc.dram_tensor("w2_16", [2, 128, FLO, D], bf16, kind="ExternalInput")
    w1_8 = (
        nc.dram_tensor("w1_8", [2, 128, 4, 4, KO2, 2, 128], fp8, kind="ExternalInput")
        if n8
        else None
    )
    w2_8 = (
        nc.dram_tensor("w2_8", [2, 128, FLO2, 2, D], fp8, kind="ExternalInput")
        if n8
        else None
    )
    b1 = nc.dram_tensor("b1", [128, 2 * FLO], fp32, kind="ExternalInput")
    y = nc.dram_tensor("y", [n_tiles, 128, KO, CT], bf16, kind="ExternalOutput")

    with TileContext(nc) as tc:
        with (
            tc.tile_pool(name="wpool", bufs=1) as wpool,
            tc.tile_pool(name="xpool", bufs=3) as xpool,
            tc.tile_pool(name="x8pool", bufs=2) as x8pool,
            tc.tile_pool(name="hpool", bufs=2) as hpool,
            tc.tile_pool(name="h8pool", bufs=2) as h8pool,
            tc.tile_pool(name="ypool", bufs=4) as ypool,
            tc.tile_pool(name="ph", bufs=4, space="PSUM") as phpool,
            tc.tile_pool(name="py", bufs=4, space="PSUM") as pypool,
        ):
            # Single-slot weight regions, reused A -> B.
            w1s = wpool.tile([128, 4, 4, KO, 128], bf16)
            w2s = wpool.tile([128, FLO, D], bf16)
            w1s8 = wpool.tile([128, 4, 4, KO2, 2, 128], fp8)
            w2s8 = wpool.tile([128, FLO2, 2, D], fp8)
            b1_sb = wpool.tile([128, 2 * FLO], fp32)

            x_first = xpool.tile([128, KO, CT], bf16, tag="x16_sb")
            # HAM warmup: ~8 dummy matmuls on a memset tile while the first
            # DMAs are in flight, so the PE clock is at 2.4 GHz (not the
            # cold 1.2 GHz) when real work arrives.
            warm = wpool.tile([128, CT], bf16)
            nc.gpsimd.memset(warm[:], 0)
            wps = phpool.tile([128, CT], fp32, tag="ph")
            for wi in range(8):
                nc.tensor.matmul(
                    wps[:],
                    lhsT=warm[:, :128],
                    rhs=warm[:],
                    start=(wi == 0),
                    stop=(wi == 7),
                )
            # Startup: first mm1 column block needs x16[0] + w1_16 A [q0,fq0].
            nc.sync.dma_start(x_first[:, 0], x16[0][:, 0])
            nc.sync.dma_start(w1s[:, 0, 0], w1_16[0, :, 0, 0])
            nc.sync.dma_start(b1_sb[:], b1[:])
            for fq in range(1, 4):
                nc.sync.dma_start(x_first[:, fq], x16[0][:, fq])
                nc.sync.dma_start(w1s[:, 0, fq], w1_16[0, :, 0, fq])
            for ko in range(4, KO):
                nc.sync.dma_start(x_first[:, ko], x16[0][:, ko])
                nc.sync.dma_start(w1s[:, 1, ko - 4], w1_16[0, :, 1, ko - 4])
            for q in range(2, 4):
                nc.sync.dma_start(w1s[:, q], w1_16[0, :, q])
            for b in range(4):
                nc.sync.dma_start(
                    w2s[:, b * 4 : (b + 1) * 4], w2_16[0, :, b * 4 : (b + 1) * 4]
                )
            # Slot A fp8 weights (used in phase A8, prefetched under A16).
            if n8:
                nc.sync.dma_start(w1s8[:], w1_8[0])
                nc.sync.dma_start(w2s8[:], w2_8[0])

            i16 = 0
            i8 = 0
            for ti, (kind, s, off, tw) in enumerate(spec):
                if kind == 0 and s == 1 and off == 0:
                    # Entering phase B16: slot B bf16 weights into the shared
                    # region. WAR deps on A16's matmuls order these after the
                    # last A16 reader; they stream during phase A8.
                    for q in range(4):
                        nc.sync.dma_start(w1s[:, q], w1_16[1, :, q])
                    for b in range(2):
                        nc.sync.dma_start(
                            w2s[:, b * 8 : (b + 1) * 8],
                            w2_16[1, :, b * 8 : (b + 1) * 8],
                        )
                if kind == 1 and s == 1 and off == 0:
                    # Entering phase B8: slot B fp8 weights (prefetch under
                    # B16, ordered after A8's readers).
                    nc.sync.dma_start(w1s8[:], w1_8[1])
                    nc.sync.dma_start(w2s8[:], w2_8[1])

                if kind == 0:
                    if i16 == 0:
                        x_sb = x_first
                    else:
                        x_sb = xpool.tile([128, KO, CT], bf16, tag="x16_sb")
                        nc.sync.dma_start(x_sb[:], x16[i16])
                    i16 += 1
                    h_sb = hpool.tile([128, FLO, CT], bf16)
                    for fo in range(FLO):
                        q, fq = divmod(fo, 4)
                        ph = phpool.tile([128, CT], fp32, tag="ph")
                        for ko in range(KO):
                            nc.tensor.matmul(
                                ph[:, :tw],
                                lhsT=w1s[:, q, fq, ko],
                                rhs=x_sb[:, ko, :tw],
                                start=(ko == 0),
                                stop=(ko == KO - 1),
                            )
                        nc.scalar.activation(
                            h_sb[:, fo, :tw],
                            ph[:, :tw],
                            (mybir.ActivationFunctionType.Identity if ACT_IDENTITY else mybir.ActivationFunctionType.Gelu),
                            bias=b1_sb[:, s * FLO + fo : s * FLO + fo + 1],
                        )
                    for do in range(KO):
                        py = pypool.tile([128, CT], fp32, tag="py")
                        for fo in range(FLO):
                            nc.tensor.matmul(
                                py[:, :tw],
                                lhsT=w2s[:, fo, do * 128 : (do + 1) * 128],
                                rhs=h_sb[:, fo, :tw],
                                start=(fo == 0),
                                stop=(fo == FLO - 1),
                            )
                        y_do = ypool.tile([128, CT], bf16, tag="y_do")
                        nc.vector.tensor_copy(y_do[:, :tw], py[:, :tw])
                        nc.sync.dma_start(y[ti][:, do, :tw], y_do[:, :tw])
                else:
                    x_sb = x8pool.tile([128, KO2, 2, CT], fp8, tag="x8_sb")
                    nc.sync.dma_start(x_sb[:], x8[i8])
                    i8 += 1
                    h_sb = h8pool.tile([128, FLO2, 2, CT], fp8)
                    for fo in range(FLO):
                        q, fq = divmod(fo, 4)
                        ph = phpool.tile([128, CT], fp32, tag="ph")
                        for j in range(KO2):
                            nc.tensor.matmul(
                                ph[:, :tw],
                                lhsT=w1s8[:, q, fq, j],
                                rhs=x_sb[:, j, :, :tw],
                                start=(j == 0),
                                stop=(j == KO2 - 1),
                                perf_mode=DR,
                            )
                        nc.scalar.activation(
                            h_sb[:, fo // 2, fo % 2, :tw],
                            ph[:, :tw],
                            (mybir.ActivationFunctionType.Identity if ACT_IDENTITY else mybir.ActivationFunctionType.Gelu),
                            bias=b1_sb[:, s * FLO + fo : s * FLO + fo + 1],
                            scale=1.0 / (SX * SW1),
                        )
                    for do in range(KO):
                        py = pypool.tile([128, CT], fp32, tag="py")
                        for g in range(FLO2):
                            nc.tensor.matmul(
                                py[:, :tw],
                                lhsT=w2s8[:, g, :, do * 128 : (do + 1) * 128],
                                rhs=h_sb[:, g, :, :tw],
                                start=(g == 0),
                                stop=(g == FLO2 - 1),
                                perf_mode=DR,
                            )
                        y_do = ypool.tile([128, CT], bf16, tag="y_do")
                        nc.vector.tensor_copy(y_do[:, :tw], py[:, :tw])
                        nc.sync.dma_start(y[ti][:, do, :tw], y_do[:, :tw])

    nc.compile()
    return nc, spec


def kernel(x, gate_w, w1, b1, w2, b2):
    from concourse.bass_utils import run_bass_kernel_spmd

    global LAST_RESULTS

    x = np.asarray(x, dtype=np.float32)
    gate_w = np.asarray(gate_w, dtype=np.float32)
    w1 = np.asarray(w1, dtype=np.float32)
    b1 = np.asarray(b1, dtype=np.float32)
    w2 = np.asarray(w2, dtype=np.float32)
    b2 = np.asarray(b2, dtype=np.float32)

    B, S, Din = x.shape
    T = B * S
    xf = x.reshape(T, D)

    # ---- Host router ----
    logits = xf.astype(np.float64) @ gate_w.astype(np.float64)
    idx0 = np.argmax(logits, axis=1)
    rows = np.arange(T)
    v0 = logits[rows, idx0]
    l2 = logits.copy()
    l2[rows, idx0] = -np.inf
    idx1 = np.argmax(l2, axis=1)
    v1_ = l2[rows, idx1]
    e1 = np.exp(v1_ - v0)
    cw0 = 1.0 / (1.0 + e1)
    cw1 = e1 / (1.0 + e1)

    # Per expert: tokens sorted by combine weight ascending; the first K8
    # go to the fp8 path.
    token_ids = []
    combine_w = []
    for e in range(E):
        sel0 = idx0 == e
        sel1 = idx1 == e
        ids = np.nonzero(sel0 | sel1)[0]
        w = np.where(sel0[ids], cw0[ids], cw1[ids])
        o = np.argsort(w)
        token_ids.append(ids[o])
        combine_w.append(w[o])

    counts = np.array([len(ids) for ids in token_ids])
    order = np.argsort(-counts)
    pairs = [(int(order[i]), int(order[E - 1 - i])) for i in range(E // 2)]
    CA = int(max(counts[eA] for eA, _ in pairs))
    CB = int(max(counts[eB] for _, eB in pairs))
    # Uniform fp8 share: DR tiles must be full 512 wide (narrower DR
    # matmuls run at the 213 ns LDWEIGHTS cadence), so K8 = 512 exactly.
    K8A = K8B = AVG_K8
    CA16 = CA - K8A
    CB16 = CB - K8B
    CA16 += CA16 & 1
    CB16 += CB16 & 1
    # per-expert fp8 counts
    K8e = np.zeros(E, dtype=int)
    for a, b in pairs:
        K8e[a] = min(max(counts[a] - CA16, 0), K8A, counts[a])
        K8e[b] = min(max(counts[b] - CB16, 0), K8B, counts[b])

    key = (CA16, CB16, K8A, K8B)
    if key not in _NC_CACHE:
        _NC_CACHE[key] = _build(CA16, CB16, K8A, K8B)
    nc, spec = _NC_CACHE[key]
    n_tiles = len(spec)

    xq = np.asarray(xf * SX, dtype=FP8)

    in_maps = [None] * N_CORES
    for pi, (eA, eB) in enumerate(pairs):
        n16 = sum(1 for k, _, _, _ in spec if k == 0)
        n8 = n_tiles - n16
        x16t = np.zeros((n16, 128, KO, CT), dtype=BF16)
        x8t = np.zeros((n8, 128, KO2, 2, CT), dtype=FP8)
        i16 = i8 = 0
        for ti, (kind, s, off, tw) in enumerate(spec):
            e = (eA, eB)[s]
            # bf16 tokens start at K8e[e] in the sorted order; fp8 at 0
            # (clipped to the expert's actual fp8 count).
            if kind == 0:
                lo, hi = K8e[e] + off, K8e[e] + off + tw
            else:
                lo, hi = off, min(off + tw, K8e[e])
            ids_seg = token_ids[e][lo:hi]
            w_val = len(ids_seg)
            if kind == 0:
                if w_val:
                    blk = (
                        xf[ids_seg]
                        .astype(BF16)
                        .reshape(w_val, KO, 128)
                        .transpose(2, 1, 0)
                    )
                    x16t[i16, :, :, :w_val] = blk
                i16 += 1
            else:
                if w_val:
                    blk = (
                        xq[ids_seg].reshape(w_val, KO2, 2, 128).transpose(3, 1, 2, 0)
                    )
                    x8t[i8, :, :, :, :w_val] = blk
                i8 += 1
        n8_nonzero = n8 > 0
        for h in range(2):
            sl = slice(h * FL, (h + 1) * FL)
            w1c16 = np.stack(
                [
                    w1[e][:, sl]
                    .reshape(KO, 128, 4, 4, 128)
                    .transpose(1, 2, 3, 0, 4)
                    for e in (eA, eB)
                ]
            ).astype(BF16)
            w2c16 = np.stack(
                [
                    w2[e][sl, :].reshape(FLO, 128, D).transpose(1, 0, 2)
                    for e in (eA, eB)
                ]
            ).astype(BF16)
            w1c8 = np.stack(
                [
                    np.asarray(w1[e][:, sl] * SW1, dtype=FP8)
                    .reshape(KO2, 2, 128, 4, 4, 128)
                    .transpose(2, 3, 4, 0, 1, 5)
                    for e in (eA, eB)
                ]
            )
            w2c8 = np.stack(
                [
                    np.asarray(w2[e][sl, :] * SW2, dtype=FP8)
                    .reshape(FLO2, 2, 128, D)
                    .transpose(2, 0, 1, 3)
                    for e in (eA, eB)
                ]
            )
            b1c = np.concatenate(
                [b1[e][sl].reshape(FLO, 128).T for e in (eA, eB)], axis=1
            )
            im = {
                "x16": np.ascontiguousarray(x16t),
                "w1_16": np.ascontiguousarray(w1c16),
                "w2_16": np.ascontiguousarray(w2c16),
                "b1": np.ascontiguousarray(b1c),
            }
            if n8:
                im["x8"] = np.ascontiguousarray(x8t)
                im["w1_8"] = np.ascontiguousarray(w1c8)
                im["w2_8"] = np.ascontiguousarray(w2c8)
            in_maps[2 * pi + h] = im

    res = run_bass_kernel_spmd(nc, in_maps, core_ids=list(range(N_CORES)))
    LAST_RESULTS = res

    out = np.zeros((T, D), dtype=np.float32)
    for pi, (eA, eB) in enumerate(pairs):
        ysum = res.results[2 * pi]["y"].astype(np.float32) + res.results[
            2 * pi + 1
        ]["y"].astype(np.float32)
        for ti, (kind, s, off, tw) in enumerate(spec):
            e = (eA, eB)[s]
            if kind == 0:
                lo, hi = K8e[e] + off, K8e[e] + off + tw
            else:
                lo, hi = off, min(off + tw, K8e[e])
            ids_seg = token_ids[e][lo:hi]
            w_val = len(ids_seg)
            if w_val == 0:
                continue
            cw_seg = combine_w[e][lo : lo + w_val].astype(np.float32)
            yt = ysum[ti, :, :, :w_val].transpose(2, 1, 0).reshape(w_val, D)
            if kind == 1:
                yt = yt / SW2
            out[ids_seg] += cw_seg[:, None] * (yt + b2[e])

    return out.reshape(B, S, D)
